# revision 1
# baseline (speedup 1.0000x reference)
"""JacobiGNN Trainium2 kernel: out = log_softmax(U @ (H * (U^T z)), axis=1).

Sharding: column-shard U across 8 cores (1024 spectral cols each). Per core,
U is streamed from DRAM exactly once; each 128x128 tile is loaded into the PE
array once as stationary weights, against which we stream both z (-> G = U^T z
contribution) and the identity (-> transposed tile in PSUM, copied to SBUF for
the second GEMM). out^T partials accumulate in PSUM packed 4x16 partitions;
ReduceScatter sums partials across cores, log_softmax runs on the local shard.
"""

import os
import sys

import numpy as np

for _p in ("/opt/trn_rl_repo", "/root/.axon_site/_ro/trn_rl_repo"):
    if os.path.isdir(_p) and _p not in sys.path:
        sys.path.insert(0, _p)

import concourse.bacc as bacc
import concourse.bass as bass  # noqa: F401
import concourse.mybir as mybir
import concourse.tile as tile
from concourse.bass_utils import run_bass_kernel_spmd

F32 = mybir.dt.float32
F32R = mybir.dt.float32r
N, F_IN, HID, C, K = 8192, 512, 64, 16, 10
BASE_ALPHA = 0.5
JA, JB, JL, JR = 1.0, 1.0, -1.0, 1.0
NCORES = 8
SH = N // NCORES      # spectral columns per core (1024)
NB = SH // 128        # column blocks per core (8)
RCH = N // 128        # row chunks (64)
MYR = SH // 128       # local row chunks (8)

_CACHE = {}


def _jacobi_coef_rows(temp):
    """Host-precomputed per-channel coefficient rows, [30*C] packed."""
    a, b, l, r = JA, JB, JL, JR
    alphas = (BASE_ALPHA * np.tanh(np.asarray(temp, np.float64)))  # [C, K+1]
    rows = [alphas[:, 0]]
    coef1 = (a - b) / 2 - (a + b + 2) / 2 * (l + r) / (r - l)
    coef2 = (a + b + 2) / (r - l)
    rows.append(coef1 * alphas[:, 1])   # c1_0
    rows.append(coef2 * alphas[:, 1])   # c1_1
    for L in range(2, K + 1):
        coef_l = 2 * L * (L + a + b) * (2 * L - 2 + a + b)
        c_lm1_1 = (2 * L + a + b - 1) * (2 * L + a + b) * (2 * L + a + b - 2)
        c_lm1_2 = (2 * L + a + b - 1) * (a ** 2 - b ** 2)
        c_lm2 = 2 * (L - 1 + a) * (L - 1 + b) * (2 * L + a + b)
        tmp1 = alphas[:, L - 1] * (c_lm1_1 / coef_l)
        tmp2 = alphas[:, L - 1] * (c_lm1_2 / coef_l)
        tmp3 = alphas[:, L - 1] * alphas[:, L - 2] * (c_lm2 / coef_l)
        rows.append(tmp1 * (2 / (r - l)))                    # t1
        rows.append(tmp1 * ((r + l) / (r - l)) + tmp2)       # t2
        rows.append(tmp3)                                    # t3
    packed = np.concatenate(rows).astype(np.float32).reshape(1, 30 * C)
    return np.ascontiguousarray(np.repeat(packed, 128, axis=0))


def _bc(ap, shape, axis=1):
    """Broadcast an AP to a 3D [128, NB, C]-style shape with stride-0 dims."""
    while ap.ndim < len(shape):
        ap = ap.unsqueeze(axis)
    return ap.broadcast_to(shape)


def _build():
    nc = bacc.Bacc("TRN2", target_bir_lowering=False, debug=False)

    u_sh = nc.dram_tensor("u_shard", [N, SH], F32R, kind="ExternalInput")
    x_sh = nc.dram_tensor("x_shard", [F_IN, SH], F32, kind="ExternalInput")
    e_sh = nc.dram_tensor("e_shard", [MYR, 128], F32, kind="ExternalInput")
    w1r = nc.dram_tensor("w1r", [128, 4 * HID], F32, kind="ExternalInput")
    w2d = nc.dram_tensor("w2d", [HID, C], F32, kind="ExternalInput")
    b1c = nc.dram_tensor("b1c", [HID, 1], F32, kind="ExternalInput")
    b2c = nc.dram_tensor("b2c", [C, 1], F32, kind="ExternalInput")
    jcd = nc.dram_tensor("jcd", [128, 30 * C], F32, kind="ExternalInput")
    id128d = nc.dram_tensor("id128d", [128, 128], F32R, kind="ExternalInput")
    id16x4d = nc.dram_tensor("id16x4d", [128, C], F32, kind="ExternalInput")
    out_sh = nc.dram_tensor("out_shard", [SH, C], F32, kind="ExternalOutput")

    rg = [list(range(NCORES))]

    with nc.allow_low_precision(reason="f32r single-pass matmul path"), \
         tile.TileContext(nc) as tc:
        with (
            tc.tile_pool(name="dram", bufs=1, space="DRAM") as dram,
            tc.tile_pool(name="consts", bufs=1) as cp,
            tc.tile_pool(name="persist", bufs=1) as pp,
            tc.tile_pool(name="xsb", bufs=2) as xp,
            tc.tile_pool(name="usb", bufs=4) as up,
            tc.tile_pool(name="utsb", bufs=2) as utp,
            tc.tile_pool(name="small", bufs=4) as sp,
        ):
            z_bounce = dram.tile([SH, C], F32R)
            z_full = dram.tile([N, C], F32R, addr_space="Shared")
            rs_in_a = dram.tile([N, C], F32)
            rs_in_b = dram.tile([N, C], F32)
            rs_out_a = dram.tile([SH, C], F32)
            rs_out_b = dram.tile([SH, C], F32)

            id128 = cp.tile_from(id128d[:])
            id16x4 = cp.tile_from(id16x4d[:])
            jc = cp.tile_from(jcd[:])
            w1 = cp.tile_from(w1r[:])
            w2 = cp.tile_from(w2d[:])
            b1 = cp.tile_from(b1c[:])
            b2 = cp.tile_from(b2c[:])
            e_row = cp.tile_from(e_sh[:])

            # ---- persistent SBUF ----
            zid = pp.tile([128, RCH, C + 128], F32R)  # [z_chunk | I128] per row chunk
            gacc = pp.tile([128, NB, C], F32)        # G = U^T z, per block
            xT = pp.tile([128, 4, SH], F32)         # x^T for the MLP
            h_sb = pp.tile([HID, SH], F32)
            zT = pp.tile([C, SH], F32)
            zme = pp.tile([128, MYR, C], F32R)       # this core's z rows
            e_col = pp.tile([128, MYR], F32)
            hacc = pp.tile([128, NB, C], F32)       # Jacobi filter H
            xs_a = pp.tile([128, NB, C], F32)
            xs_b = pp.tile([128, NB, C], F32)
            htmp = pp.tile([128, NB, C], F32)
            htmp2 = pp.tile([128, NB, C], F32)
            accsb_a = pp.tile([128, 2048], F32)      # out^T blocks 0-3
            accsb_b = pp.tile([128, 2048], F32)      # out^T blocks 4-7
            smin = pp.tile([128, MYR, C], F32)
            smb = pp.tile([128, MYR, C], F32)
            smout = pp.tile([128, MYR, C], F32)
            # identity halves of zid (z part DMA'd after the allgather)
            nc.gpsimd.dma_start(
                out=zid[:, :, C:C + 128],
                in_=id128[:].unsqueeze(1).broadcast_to((128, RCH, 128)))

            # ================= phase 0: MLP head -> z, allgather =========
            with tc.tile_pool(name="ppre", bufs=1, space="PSUM") as ppre:
                nc.scalar.dma_start(
                    out=xT[:], in_=x_sh[:].rearrange("(a p) r -> p a r", p=128))
                ph = ppre.tile([HID, SH], F32, tag="ph")
                for half in range(2):
                    for fb in range(4):
                        nc.tensor.matmul(
                            ph[:, half * 512:(half + 1) * 512],
                            lhsT=w1[:, fb * HID:(fb + 1) * HID],
                            rhs=xT[:, fb, half * 512:(half + 1) * 512],
                            start=(fb == 0), stop=(fb == 3),
                        )
                nc.scalar.activation(h_sb[:], ph[:], mybir.ActivationFunctionType.Relu,
                                     bias=b1[:, 0:1], scale=1.0)
                pz = ppre.tile([C, SH], F32, tag="pz")
                for half in range(2):
                    nc.tensor.matmul(
                        pz[:, half * 512:(half + 1) * 512],
                        lhsT=w2[:], rhs=h_sb[:, half * 512:(half + 1) * 512],
                        start=True, stop=True,
                    )
                nc.vector.tensor_scalar_add(zT[:], pz[:], b2[:, 0:1])
                # e: [8, 128] -> [128, 8]
                pet = ppre.tile([128, MYR], F32, tag="ptmp", bufs=3)
                nc.tensor.transpose(pet[:], e_row[:], id16x4[0:MYR, 0:MYR])
                nc.scalar.copy(e_col[:], pet[:])
                # z^T -> z rows for this core
                for rc in range(MYR):
                    pzt = ppre.tile([128, C], F32, tag="ptmp", bufs=3)
                    nc.tensor.transpose(pzt[:], zT[:, rc * 128:(rc + 1) * 128], id16x4[0:C, :])
                    nc.scalar.copy(zme[:, rc, :], pzt[:])
                nc.scalar.dma_start(
                    out=z_bounce[:].rearrange("(j p) c -> p j c", p=128), in_=zme[:])
            nc.gpsimd.collective_compute(
                "AllGather", mybir.AluOpType.bypass, replica_groups=rg,
                ins=[z_bounce.opt()], outs=[z_full.opt()],
            )
            nc.scalar.dma_start(
                out=zid[:, :, 0:C],
                in_=z_full[:].rearrange("(j p) c -> p j c", p=128))

            # ================= Jacobi filter H on DVE ====================
            ev = _bc(e_col[:], (128, NB, C), axis=2)

            def jrow(i):
                return _bc(jc[:, i * C:(i + 1) * C], (128, NB, C))

            nc.vector.tensor_copy(xs_a[:], jrow(0))                       # xs_m2
            nc.vector.tensor_mul(htmp[:], xs_a[:], ev)
            nc.vector.tensor_mul(htmp[:], htmp[:], jrow(2))
            nc.vector.tensor_add(xs_b[:], htmp[:], jrow(1))               # xs_m1
            nc.vector.tensor_add(hacc[:], xs_a[:], xs_b[:])
            xm2, xm1 = xs_a, xs_b
            for L in range(2, K + 1):
                r0 = 3 + 3 * (L - 2)
                nc.vector.tensor_mul(htmp[:], xm1[:], ev)
                nc.vector.tensor_mul(htmp[:], htmp[:], jrow(r0))
                nc.vector.tensor_mul(htmp2[:], xm1[:], jrow(r0 + 1))
                nc.vector.tensor_sub(htmp[:], htmp[:], htmp2[:])
                nc.vector.tensor_mul(htmp2[:], xm2[:], jrow(r0 + 2))
                nc.vector.tensor_sub(xm2[:], htmp[:], htmp2[:])           # nx
                nc.vector.tensor_add(hacc[:], hacc[:], xm2[:])
                xm2, xm1 = xm1, xm2

            # ================= main loop over column blocks ==============
            def emit_acc_tail(pm, accsb, rs_in):
                """Transpose out^T quarters to natural rows and DMA to rs_in."""
                for rc in range(RCH):
                    kq, jj = rc // 16, rc % 16
                    pt2 = pm.tile([128, 2, C + 128], F32, tag="pt", bufs=4)
                    nc.tensor.transpose(
                        pt2[:, 0, 0:C],
                        accsb[32 * kq:32 * kq + C, jj * 128:(jj + 1) * 128],
                        id16x4[32 * kq:32 * kq + C, :],
                        tile_position=(32 * kq, 0),
                    )
                    o2 = sp.tile([128, C], F32, tag="o2")
                    if rc % 2 == 0:
                        nc.scalar.copy(o2[:], pt2[:, 0, 0:C])
                    else:
                        nc.vector.tensor_copy(o2[:], pt2[:, 0, 0:C])
                    nc.gpsimd.dma_start(out=rs_in[rc * 128:(rc + 1) * 128, :], in_=o2[:])

            with (
                tc.tile_pool(name="pmain", bufs=1, space="PSUM") as pm,
            ):
                pacc = pm.tile([128, 2048], F32, tag="pacc")       # 4 banks
                for b in range(NB):
                    ut_t = utp.tile([128, RCH, C + 128], F32, tag="ut")
                    for g in range(RCH // 8):
                        u_t = up.tile([128, 8, 128], F32R, tag="u")
                        nc.sync.dma_start(
                            out=u_t[:],
                            in_=u_sh[1024 * g:1024 * (g + 1), 128 * b:128 * (b + 1)]
                            .rearrange("(j p) c -> p j c", p=128),
                        )
                        for jj in range(8):
                            rc = 8 * g + jj
                            q = rc % 2
                            if q == 0:
                                pt = pm.tile([128, 2, C + 128], F32, tag="pt", bufs=4)
                            # one matmul: [G contrib | transposed tile]
                            nc.tensor.matmul(
                                pt[:, q, :], lhsT=u_t[:, jj, :], rhs=zid[:, rc, :],
                                start=True, stop=True,
                            )
                            if q == 1:
                                dst = ut_t[:, rc - 1:rc + 1, :]
                                if (rc // 2) % 2 == 0:
                                    nc.scalar.copy(dst, pt[:])
                                else:
                                    nc.vector.tensor_copy(dst, pt[:])
                    # G_b = sum over row chunks (reduce middle dim on DVE)
                    nc.vector.tensor_reduce(
                        out=gacc[:, b, :], in_=ut_t[:, :, 0:C].transpose([0, 2, 1]),
                        op=mybir.AluOpType.add, axis=mybir.AxisListType.X,
                    )
                    y_t = sp.tile([128, C], F32, tag="y")
                    nc.vector.tensor_mul(y_t[:], gacc[:, b, :], hacc[:, b, :])
                    for rgp in range(16):
                        kq, off = rgp // 4, (rgp % 4) * 512
                        nc.tensor.matmul(
                            pacc[32 * kq:32 * kq + C, off:off + 512],
                            lhsT=y_t[:], rhs=ut_t[:, rgp * 4:(rgp + 1) * 4, C:C + 128],
                            start=(b % 4 == 0), stop=(b % 4 == 3),
                            skip_group_check=True,
                            tile_position=(0, 32 * kq),
                        )
                    if b == 3:
                        nc.scalar.copy(accsb_a[:], pacc[:])
                        emit_acc_tail(pm, accsb_a, rs_in_a)
                        nc.gpsimd.collective_compute(
                            "ReduceScatter", mybir.AluOpType.add, replica_groups=rg,
                            ins=[rs_in_a.opt()], outs=[rs_out_a.opt()],
                        )
                    if b == 7:
                        nc.scalar.copy(accsb_b[:], pacc[:])
                        emit_acc_tail(pm, accsb_b, rs_in_b)
                        nc.gpsimd.collective_compute(
                "ReduceScatter", mybir.AluOpType.add, replica_groups=rg,
                ins=[rs_in_b.opt()], outs=[rs_out_b.opt()],
            )
            nc.sync.dma_start(out=smin[:], in_=rs_out_a[:].rearrange("(j p) c -> p j c", p=128))
            nc.sync.dma_start(out=smb[:], in_=rs_out_b[:].rearrange("(j p) c -> p j c", p=128))
            nc.vector.tensor_add(smin[:], smin[:], smb[:])
            for rc in range(MYR):
                s = smin[:, rc, :]
                mneg = sp.tile([128, 1], F32, tag="mneg")
                ssum = sp.tile([128, 1], F32, tag="ssum")
                lns = sp.tile([128, 1], F32, tag="lns")
                et = sp.tile([128, C], F32, tag="et")
                nc.vector.tensor_reduce(out=mneg[:], in_=s, op=mybir.AluOpType.max,
                                        axis=mybir.AxisListType.X, negate=True)
                nc.scalar.activation(et[:], s, mybir.ActivationFunctionType.Exp,
                                     bias=mneg[:, 0:1], scale=1.0, accum_out=ssum[:, 0:1])
                nc.scalar.activation(lns[:], ssum[:], mybir.ActivationFunctionType.Ln)
                nc.vector.tensor_scalar(
                    out=smout[:, rc, :], in0=s, scalar1=mneg[:, 0:1], scalar2=lns[:, 0:1],
                    op0=mybir.AluOpType.add, op1=mybir.AluOpType.subtract,
                )
            nc.scalar.dma_start(
                out=out_sh[:].rearrange("(j p) c -> p j c", p=128), in_=smout[:])

    nc.compile()
    return nc


def _prep_inputs(origin_e, U, x, W1, b1, W2, b2, temp):
    origin_e = np.ascontiguousarray(np.asarray(origin_e, np.float32))
    U = np.asarray(U, np.float32)
    x = np.asarray(x, np.float32)
    W1 = np.asarray(W1, np.float32)
    b1 = np.asarray(b1, np.float32)
    W2 = np.asarray(W2, np.float32)
    b2 = np.asarray(b2, np.float32)

    jc = _jacobi_coef_rows(temp)
    id128 = np.eye(128, dtype=np.float32)
    id16 = np.zeros((128, C), np.float32)
    for k in range(4):
        id16[32 * k:32 * k + C, :] = np.eye(C, dtype=np.float32)
    w1r = np.ascontiguousarray(
        W1.reshape(4, 128, HID).transpose(1, 0, 2).reshape(128, 4 * HID))
    shared = {
        "w1r": w1r, "w2d": np.ascontiguousarray(W2),
        "b1c": np.ascontiguousarray(b1.reshape(HID, 1)),
        "b2c": np.ascontiguousarray(b2.reshape(C, 1)),
        "jcd": jc, "id128d": id128, "id16x4d": id16,
    }
    in_maps = []
    for i in range(NCORES):
        m = dict(shared)
        m["u_shard"] = np.ascontiguousarray(U[:, i * SH:(i + 1) * SH])
        m["x_shard"] = np.ascontiguousarray(x[i * SH:(i + 1) * SH, :].T)
        m["e_shard"] = np.ascontiguousarray(
            origin_e[i * SH:(i + 1) * SH].reshape(MYR, 128))
        in_maps.append(m)
    return in_maps


def _get_program():
    if "nc" not in _CACHE:
        _CACHE["nc"] = _build()
    return _CACHE["nc"]


def run(inputs, trace=False, **kw):
    nc = _get_program()
    in_maps = _prep_inputs(**inputs)
    res = run_bass_kernel_spmd(nc, in_maps, core_ids=list(range(NCORES)),
                               trace=trace, **kw)
    out = np.concatenate([res.results[i]["out_shard"] for i in range(NCORES)], axis=0)
    return out, res


def kernel(origin_e, U, x, W1, b1, W2, b2, temp):
    out, _ = run(dict(origin_e=origin_e, U=U, x=x, W1=W1, b1=b1, W2=W2,
                      b2=b2, temp=temp))
    return out



# revision 30
# speedup vs baseline: 2.5837x; 2.5837x over previous
"""JacobiGNN Trainium2 kernel: out = log_softmax(U @ (H * (U^T z)), axis=1).

v2: bf16 datapath. Column-shard U across 8 cores (1024 spectral cols each).
Per 128-col block b: each 128x128 U tile is loaded once as PE weights; we
stream z (16 cols, accumulating G_b = U_b^T z in f32 PSUM) and the identity
(128 cols, materializing the transposed tile -> SBUF bf16). GEMM2 then uses
the transposed tiles as weights and streams y_b = H_b * G_b (16 cols),
accumulating output rows in natural layout in PSUM across blocks. A single
bf16 ReduceScatter over the partition-major flat buffer sums partials across
cores; log_softmax runs on the local shard. U tiles for blocks 0-1 are
transposed early so the PE has work while the z AllGather is in flight.
"""

import os
import sys

import numpy as np

for _p in ("/opt/trn_rl_repo", "/root/.axon_site/_ro/trn_rl_repo"):
    if os.path.isdir(_p) and _p not in sys.path:
        sys.path.insert(0, _p)

import ml_dtypes

import concourse.bacc as bacc
import concourse.bass as bass  # noqa: F401
import concourse.mybir as mybir
import concourse.tile as tile
from concourse.bass_utils import run_bass_kernel_spmd

F32 = mybir.dt.float32
BF16 = mybir.dt.bfloat16
NPBF16 = ml_dtypes.bfloat16
N, F_IN, HID, C, K = 8192, 512, 64, 16, 10
BASE_ALPHA = 0.5
JA, JB, JL, JR = 1.0, 1.0, -1.0, 1.0
NCORES = 8
SH = N // NCORES      # spectral columns per core (1024)
NB = SH // 128        # column blocks per core (8)
RCH = N // 128        # row chunks (64)
MYR = SH // 128       # (8)
NPRE = 4              # blocks whose transpose sweep runs during the AllGather

_CACHE = {}
DEBUG = False


def _jacobi_coef_rows(temp):
    """Host-precomputed per-channel coefficient rows, [30*C] packed."""
    a, b, l, r = JA, JB, JL, JR
    alphas = (BASE_ALPHA * np.tanh(np.asarray(temp, np.float64)))  # [C, K+1]
    rows = [alphas[:, 0]]
    coef1 = (a - b) / 2 - (a + b + 2) / 2 * (l + r) / (r - l)
    coef2 = (a + b + 2) / (r - l)
    rows.append(coef1 * alphas[:, 1])   # c1_0
    rows.append(coef2 * alphas[:, 1])   # c1_1
    for L in range(2, K + 1):
        coef_l = 2 * L * (L + a + b) * (2 * L - 2 + a + b)
        c_lm1_1 = (2 * L + a + b - 1) * (2 * L + a + b) * (2 * L + a + b - 2)
        c_lm1_2 = (2 * L + a + b - 1) * (a ** 2 - b ** 2)
        c_lm2 = 2 * (L - 1 + a) * (L - 1 + b) * (2 * L + a + b)
        tmp1 = alphas[:, L - 1] * (c_lm1_1 / coef_l)
        tmp2 = alphas[:, L - 1] * (c_lm1_2 / coef_l)
        tmp3 = alphas[:, L - 1] * alphas[:, L - 2] * (c_lm2 / coef_l)
        rows.append(tmp1 * (2 / (r - l)))                    # t1
        rows.append(tmp1 * ((r + l) / (r - l)) + tmp2)       # t2
        rows.append(tmp3)                                    # t3
    packed = np.concatenate(rows).astype(np.float32).reshape(1, 30 * C)
    return np.ascontiguousarray(np.repeat(packed, 128, axis=0))


def _bc(ap, shape, axis=1):
    """Broadcast an AP to a 3D [128, NB, C]-style shape with stride-0 dims."""
    while ap.ndim < len(shape):
        ap = ap.unsqueeze(axis)
    return ap.broadcast_to(shape)


def _build():
    nc = bacc.Bacc("TRN2", target_bir_lowering=False, debug=False)

    u4 = nc.dram_tensor("u4", [NB, 128, RCH, 128], BF16, kind="ExternalInput")
    x3 = nc.dram_tensor("x3", [128, 4, SH], BF16, kind="ExternalInput")
    e_sh = nc.dram_tensor("e_shard", [MYR, 128], F32, kind="ExternalInput")
    w1r = nc.dram_tensor("w1r", [128, 4 * HID], BF16, kind="ExternalInput")
    w2d = nc.dram_tensor("w2d", [HID, C], BF16, kind="ExternalInput")
    b1c = nc.dram_tensor("b1c", [HID, 1], F32, kind="ExternalInput")
    b2c = nc.dram_tensor("b2c", [C, 1], F32, kind="ExternalInput")
    jcd = nc.dram_tensor("jcd", [128, 30 * C], F32, kind="ExternalInput")
    id128d = nc.dram_tensor("id128d", [128, 128], BF16, kind="ExternalInput")
    id16d = nc.dram_tensor("id16d", [C, C], F32, kind="ExternalInput")
    out_sh = nc.dram_tensor("out_sh", [C, RCH, C], F32, kind="ExternalOutput")
    if DEBUG:
        dbg_zid = nc.dram_tensor("dbg_zid", [128, RCH, C], BF16, kind="ExternalOutput")
        dbg_h = nc.dram_tensor("dbg_h", [128, NB, C], F32, kind="ExternalOutput")
        dbg_g = nc.dram_tensor("dbg_g", [128, NB, C], F32, kind="ExternalOutput")
        dbg_y = nc.dram_tensor("dbg_y", [128, NB, C], BF16, kind="ExternalOutput")
        dbg_ut7 = nc.dram_tensor("dbg_ut7", [128, RCH, 128], BF16, kind="ExternalOutput")
        dbg_osb = nc.dram_tensor("dbg_osb", [128, RCH, C], BF16, kind="ExternalOutput")
        dbg_zt = nc.dram_tensor("dbg_zt", [C, SH], F32, kind="ExternalOutput")

    rg = [list(range(NCORES))]

    with nc.allow_low_precision(reason="bf16 matmul path, tol 2e-2"), \
         tile.TileContext(nc) as tc:
        with (
            tc.tile_pool(name="dram", bufs=1, space="DRAM") as dram,
            tc.tile_pool(name="consts", bufs=1) as cp,
            tc.tile_pool(name="persist", bufs=1) as pp,
            tc.tile_pool(name="usb", bufs=4) as up,
            tc.tile_pool(name="utsb", bufs=4) as utp,
            tc.tile_pool(name="small", bufs=4) as sp,
            tc.tile_pool(name="ysb", bufs=4) as yp,
        ):
            warm_in = dram.tile([C, C], BF16)
            warm_out = dram.tile([NCORES * C, C], BF16, addr_space="Shared")
            z_bounce = dram.tile([128, MYR, C], BF16)
            z_full = dram.tile([NCORES, 128, MYR, C], BF16, addr_space="Shared")
            rs_in = dram.tile([128, RCH, C], BF16)
            rs_out = dram.tile([C, RCH, C], BF16)

            id128 = cp.tile_from(id128d[:])
            id16 = cp.tile_from(id16d[:])
            jc = cp.tile_from(jcd[:])
            w1 = cp.tile_from(w1r[:])
            w2 = cp.tile_from(w2d[:])
            b1 = cp.tile_from(b1c[:])
            b2 = cp.tile_from(b2c[:])
            e_row = cp.tile_from(e_sh[:])

            # ---- persistent SBUF ----
            xT = pp.tile([128, 4, SH], BF16)
            h_sb = pp.tile([HID, SH], BF16)
            zT = pp.tile([C, SH], F32)
            zme = pp.tile([128, MYR, C], BF16)       # this core's z rows
            zid = pp.tile([128, RCH, C], BF16)       # full z, chunk-major
            e_col = pp.tile([128, MYR], F32)
            hacc = pp.tile([128, NB, C], F32)        # Jacobi filter H
            xs_a = pp.tile([128, NB, C], F32)
            xs_b = pp.tile([128, NB, C], F32)
            htmp = pp.tile([128, NB, C], F32)
            htmp2 = pp.tile([128, NB, C], F32)
            out_sb = pp.tile([128, RCH, C], BF16)
            srs = pp.tile([C, RCH, C], BF16)
            smt = pp.tile([C, RCH, C], F32)
            smet = pp.tile([C, RCH, C], F32)
            smout = pp.tile([C, RCH, C], F32)

            # warm up the collective path: first collective pays ~15us of
            # one-time setup; absorb it during the MLP + transpose sweeps.
            nc.gpsimd.collective_compute(
                "AllGather", mybir.AluOpType.bypass, replica_groups=rg,
                ins=[warm_in.opt()], outs=[warm_out.opt()],
            )
            nc.scalar.dma_start(out=xT[:], in_=x3[:])
            # u DMAs alternate sync/scalar HWDGE queues so the zid DMA
            # (which waits on the AllGather) never blocks a u stream.
            u_tiles = []
            for b in range(NPRE):
                u_t = up.tile([128, RCH, 128], BF16, tag="u")
                eng = nc.sync if b % 2 == 0 else nc.scalar
                eng.dma_start(out=u_t[:], in_=u4[b])
                u_tiles.append(u_t)

            # ================= phase 0: MLP head -> z, allgather =========
            with tc.tile_pool(name="ppre", bufs=1, space="PSUM") as ppre:
                ph = ppre.tile([HID, SH], F32, tag="ph")
                for half in range(2):
                    for fb in range(4):
                        nc.tensor.matmul(
                            ph[:, half * 512:(half + 1) * 512],
                            lhsT=w1[:, fb * HID:(fb + 1) * HID],
                            rhs=xT[:, fb, half * 512:(half + 1) * 512],
                            start=(fb == 0), stop=(fb == 3),
                        )
                nc.scalar.activation(h_sb[:], ph[:], mybir.ActivationFunctionType.Relu,
                                     bias=b1[:, 0:1], scale=1.0)
                pz = ppre.tile([C, SH], F32, tag="pz")
                for half in range(2):
                    nc.tensor.matmul(
                        pz[:, half * 512:(half + 1) * 512],
                        lhsT=w2[:], rhs=h_sb[:, half * 512:(half + 1) * 512],
                        start=True, stop=True,
                    )
                nc.vector.tensor_scalar_add(zT[:], pz[:], b2[:, 0:1])
                # e: [8, 128] -> [128, 8]
                pet = ppre.tile([128, MYR], F32, tag="ptmp", bufs=3)
                nc.tensor.transpose(pet[:], e_row[:], id16[0:MYR, 0:MYR])
                nc.scalar.copy(e_col[:], pet[:])
                # z^T -> z rows for this core (bf16)
                for rc in range(MYR):
                    pzt = ppre.tile([128, C], F32, tag="ptmp", bufs=3)
                    nc.tensor.transpose(pzt[:], zT[:, rc * 128:(rc + 1) * 128],
                                        id16[:])
                    nc.scalar.copy(zme[:, rc, :], pzt[:])
                nc.scalar.dma_start(out=z_bounce[:], in_=zme[:])
            nc.gpsimd.collective_compute(
                "AllGather", mybir.AluOpType.bypass, replica_groups=rg,
                ins=[z_bounce.opt()], outs=[z_full.opt()],
            )
            nc.sync.dma_start(
                out=zid[:].rearrange("p (i j) c -> p i j c", i=NCORES),
                in_=z_full[:].rearrange("i p j c -> p i j c"))

            # ================= Jacobi filter H on DVE ====================
            ev = _bc(e_col[:], (128, NB, C), axis=2)

            def jrow(i):
                return _bc(jc[:, i * C:(i + 1) * C], (128, NB, C))

            nc.vector.tensor_copy(xs_a[:], jrow(0))                       # xs_m2
            nc.vector.tensor_mul(htmp[:], xs_a[:], ev)
            nc.vector.tensor_mul(htmp[:], htmp[:], jrow(2))
            nc.vector.tensor_add(xs_b[:], htmp[:], jrow(1))               # xs_m1
            nc.vector.tensor_add(hacc[:], xs_a[:], xs_b[:])
            xm2, xm1 = xs_a, xs_b
            for L in range(2, K + 1):
                r0 = 3 + 3 * (L - 2)
                nc.vector.tensor_mul(htmp[:], xm1[:], ev)
                nc.vector.tensor_mul(htmp[:], htmp[:], jrow(r0))
                nc.vector.tensor_mul(htmp2[:], xm1[:], jrow(r0 + 1))
                nc.vector.tensor_sub(htmp[:], htmp[:], htmp2[:])
                nc.vector.tensor_mul(htmp2[:], xm2[:], jrow(r0 + 2))
                nc.vector.tensor_sub(xm2[:], htmp[:], htmp2[:])           # nx
                nc.vector.tensor_add(hacc[:], hacc[:], xm2[:])
                xm2, xm1 = xm1, xm2

            # ================= main loop over column blocks ==============
            with tc.tile_pool(name="pmain", bufs=1, space="PSUM") as pm:
                oacc = pm.tile([128, RCH, C], F32, tag="oacc")   # 2 banks
                ut_tiles = {}
                if DEBUG:
                    dbg_gsb = pp.tile([128, NB, C], F32)
                    dbg_ysb = pp.tile([128, NB, C], BF16)
                nonlocal_ut = []

                def transpose_sweep(b, u_t):
                    """U tiles of block b -> transposed bf16 tiles in SBUF."""
                    ut_sb = utp.tile([128, RCH, 128], BF16, tag="ut")
                    for g in range(RCH // 8):
                        pt = pm.tile([128, 8, 128], F32, tag="pt", bufs=2)
                        for j in range(8):
                            rc = 8 * g + j
                            nc.tensor.matmul(
                                pt[:, j, :], lhsT=u_t[:, rc, :], rhs=id128[:],
                                start=True, stop=True,
                            )
                        dst = ut_sb[:, 8 * g:8 * (g + 1), :]
                        if g % 2 == 0:
                            nc.scalar.copy(dst, pt[:])
                        else:
                            nc.vector.tensor_copy(dst, pt[:])
                    ut_tiles[b] = ut_sb

                def gemm1_z(b, u_t):
                    """Accumulate G_b = U_b^T z into PSUM; returns y_b bf16."""
                    g_ps = pm.tile([128, C], F32, tag="g", bufs=2)
                    for rc in range(RCH):
                        nc.tensor.matmul(
                            g_ps[:], lhsT=u_t[:, rc, :], rhs=zid[:, rc, :],
                            start=(rc == 0), stop=(rc == RCH - 1),
                            skip_group_check=True,
                        )
                    y_sb = yp.tile([128, C], BF16, tag="y")
                    nc.vector.tensor_mul(y_sb[:], g_ps[:], hacc[:, b, :])
                    if DEBUG:
                        nc.scalar.copy(dbg_gsb[:, b, :], g_ps[:])
                        nc.vector.tensor_copy(dbg_ysb[:, b, :], y_sb[:])
                    return y_sb

                def gemm1_both(b, u_t):
                    """Interleaved transpose + z-accumulate sweep (shared weights)."""
                    ut_sb = utp.tile([128, RCH, 128], BF16, tag="ut")
                    g_ps = pm.tile([128, C], F32, tag="g", bufs=2)
                    for g in range(RCH // 8):
                        pt = pm.tile([128, 8, 128], F32, tag="pt", bufs=2)
                        for j in range(8):
                            rc = 8 * g + j
                            nc.tensor.matmul(
                                g_ps[:], lhsT=u_t[:, rc, :], rhs=zid[:, rc, :],
                                start=(rc == 0), stop=(rc == RCH - 1),
                                skip_group_check=True,
                            )
                            nc.tensor.matmul(
                                pt[:, j, :], lhsT=u_t[:, rc, :], rhs=id128[:],
                                start=True, stop=True,
                            )
                        dst = ut_sb[:, 8 * g:8 * (g + 1), :]
                        if g % 2 == 0:
                            nc.scalar.copy(dst, pt[:])
                        else:
                            nc.vector.tensor_copy(dst, pt[:])
                    ut_tiles[b] = ut_sb
                    y_sb = yp.tile([128, C], BF16, tag="y")
                    nc.vector.tensor_mul(y_sb[:], g_ps[:], hacc[:, b, :])
                    if DEBUG:
                        nc.scalar.copy(dbg_gsb[:, b, :], g_ps[:])
                        nc.vector.tensor_copy(dbg_ysb[:, b, :], y_sb[:])
                    return y_sb

                def gemm2(b, y_sb):
                    """out rows += Ut_b^T y_b, natural layout, accumulate over b."""
                    ut_sb = ut_tiles.pop(b)
                    if DEBUG and b == NB - 1:
                        nonlocal_ut.append(ut_sb)
                    for rc in range(RCH):
                        # start=True pends-zero the WHOLE 2KB psum zero region
                        # (bank); only the first write per bank may set it, the
                        # rest of b==0's writes land on pending-zero bytes and
                        # overwrite, b>0 accumulates.
                        nc.tensor.matmul(
                            oacc[:, rc, :], lhsT=ut_sb[:, rc, :], rhs=y_sb[:],
                            start=(b == 0 and rc % 32 == 0), stop=(b == NB - 1),
                            skip_group_check=True,
                        )

                # blocks 0..NPRE-1: transpose sweeps run during the AllGather
                for b in range(NPRE):
                    transpose_sweep(b, u_tiles[b])
                ys = {}
                for b in range(NPRE):
                    ys[b] = gemm1_z(b, u_tiles[b])
                # u DMAs for b>=NPRE are emitted only now so their buffer-reuse
                # WAR deps cover the gemm1_z reads of blocks 0..NPRE-1
                for b in range(NPRE, NB):
                    u_t = up.tile([128, RCH, 128], BF16, tag="u")
                    eng = nc.sync if b % 2 == 0 else nc.scalar
                    eng.dma_start(out=u_t[:], in_=u4[b])
                    u_tiles.append(u_t)
                # software pipeline: one pending GEMM2 interleaves with each GEMM1
                gemm2(0, ys.pop(0))
                nxt = 1
                for b in range(NPRE, NB):
                    ys[b] = gemm1_both(b, u_tiles[b])
                    gemm2(nxt, ys.pop(nxt))
                    nxt += 1
                while nxt < NB:
                    gemm2(nxt, ys.pop(nxt))
                    nxt += 1
                if DEBUG:
                    nc.scalar.dma_start(out=dbg_zid[:], in_=zid[:])
                    nc.scalar.dma_start(out=dbg_h[:], in_=hacc[:])
                    nc.scalar.dma_start(out=dbg_g[:], in_=dbg_gsb[:])
                    nc.scalar.dma_start(out=dbg_y[:], in_=dbg_ysb[:])
                    nc.scalar.dma_start(out=dbg_ut7[:], in_=nonlocal_ut[0][:])
                    nc.scalar.dma_start(out=dbg_zt[:], in_=zT[:])

                # flush -> bf16 -> ReduceScatter on partition-major buffer
                nc.scalar.copy(out_sb[:, 0:RCH // 2, :], oacc[:, 0:RCH // 2, :])
                nc.vector.tensor_copy(out_sb[:, RCH // 2:, :], oacc[:, RCH // 2:, :])
                if DEBUG:
                    nc.scalar.dma_start(out=dbg_osb[:], in_=out_sb[:])
                nc.sync.dma_start(out=rs_in[:], in_=out_sb[:])
            nc.gpsimd.collective_compute(
                "ReduceScatter", mybir.AluOpType.add, replica_groups=rg,
                ins=[rs_in.opt()], outs=[rs_out.opt()],
            )
            # log_softmax on the local [16, 64, 16] shard: single Exp + single
            # Ln (one activation-table load each), per-row stats via DVE.
            nc.sync.dma_start(out=srs[:], in_=rs_out[:])
            mneg = sp.tile([C, RCH, 1], F32, tag="mneg")
            ssum = sp.tile([C, RCH, 1], F32, tag="ssum")
            lns = sp.tile([C, RCH, 1], F32, tag="lns")
            nc.vector.tensor_reduce(out=mneg[:], in_=srs[:], op=mybir.AluOpType.max,
                                    axis=mybir.AxisListType.X, negate=True)
            nc.vector.tensor_add(smt[:], srs[:], mneg[:].broadcast_to((C, RCH, C)))
            nc.scalar.activation(smet[:], smt[:], mybir.ActivationFunctionType.Exp)
            nc.vector.tensor_reduce(out=ssum[:], in_=smet[:], op=mybir.AluOpType.add,
                                    axis=mybir.AxisListType.X)
            nc.scalar.activation(lns[:], ssum[:], mybir.ActivationFunctionType.Ln)
            nc.vector.tensor_sub(smout[:], smt[:], lns[:].broadcast_to((C, RCH, C)))
            nc.scalar.dma_start(out=out_sh[:], in_=smout[:])

    nc.compile()
    return nc


def _prep_inputs(origin_e, U, x, W1, b1, W2, b2, temp):
    origin_e = np.ascontiguousarray(np.asarray(origin_e, np.float32))
    U = np.asarray(U, np.float32)
    x = np.asarray(x, np.float32)
    W1 = np.asarray(W1, np.float32)
    b1 = np.asarray(b1, np.float32)
    W2 = np.asarray(W2, np.float32)
    b2 = np.asarray(b2, np.float32)

    jc = _jacobi_coef_rows(temp)
    id128 = np.eye(128, dtype=NPBF16)
    id16 = np.eye(C, dtype=np.float32)
    w1r = np.ascontiguousarray(
        W1.reshape(4, 128, HID).transpose(1, 0, 2).reshape(128, 4 * HID)
        .astype(NPBF16))
    shared = {
        "w1r": w1r, "w2d": np.ascontiguousarray(W2.astype(NPBF16)),
        "b1c": np.ascontiguousarray(b1.reshape(HID, 1)),
        "b2c": np.ascontiguousarray(b2.reshape(C, 1)),
        "jcd": jc, "id128d": id128, "id16d": id16,
    }
    Ub = U.astype(NPBF16)
    xb = x.astype(NPBF16)
    in_maps = []
    for i in range(NCORES):
        m = dict(shared)
        # u4[b, p, rc, c] = U[rc*128 + p, i*1024 + b*128 + c]
        A = Ub[:, i * SH:(i + 1) * SH]              # [8192, 1024]
        A = A.reshape(RCH, 128, NB, 128)            # [rc, p, b, c]
        m["u4"] = np.ascontiguousarray(A.transpose(2, 1, 0, 3))
        # x3[p, a, r] = x[i*1024 + r, a*128 + p]
        xs = xb[i * SH:(i + 1) * SH, :].T           # [512, 1024]
        m["x3"] = np.ascontiguousarray(
            xs.reshape(4, 128, SH).transpose(1, 0, 2))
        m["e_shard"] = np.ascontiguousarray(
            origin_e[i * SH:(i + 1) * SH].reshape(MYR, 128))
        in_maps.append(m)
    return in_maps


def _get_program():
    if "nc" not in _CACHE:
        _CACHE["nc"] = _build()
    return _CACHE["nc"]


def _unshard(res):
    """res[i]['out_sh'] is [16, 64, 16] with row = rc*128 + 16*i + p."""
    M = np.empty((RCH, 128, C), dtype=np.float32)
    for i in range(NCORES):
        arr = np.asarray(res.results[i]["out_sh"], np.float32)
        M[:, C * i:C * (i + 1), :] = arr.transpose(1, 0, 2)
    return M.reshape(N, C)


def run(inputs, trace=False, **kw):
    nc = _get_program()
    in_maps = _prep_inputs(**inputs)
    res = run_bass_kernel_spmd(nc, in_maps, core_ids=list(range(NCORES)),
                               trace=trace, **kw)
    return _unshard(res), res


def kernel(origin_e, U, x, W1, b1, W2, b2, temp):
    out, _ = run(dict(origin_e=origin_e, U=U, x=x, W1=W1, b1=b1, W2=W2,
                      b2=b2, temp=temp))
    return out
